# revision 9
# baseline (speedup 1.0000x reference)
"""Trainium2 Bass kernel: single-step ConvLSTM cell with spatial multi-head
attention (nn_CvtLstm). Data-parallel over batch N=8 across 8 NeuronCores.

Math (first timestep, h0 = c0 = 0):
  xt = tanh(w_in @ x + b_in)
  z  = conv3x3(xt, w_conv[:, :128]) + b_conv        (h-half of w_conv sees zeros)
  q/k/v = 1x1 projections of z, 8 heads x 16 ch over 1296 spatial tokens
  a  = per-head V @ softmax(q^T k, axis=keys)^T
  i, g, o = gates on [z; a]  (f-gate unused: c0 = 0)
  out = w_out @ (sigmoid(o) * tanh(sigmoid(i) * tanh(g))) + b_out

On-chip layout: feature maps [channels=128 partitions, HW=1296 free].
Attention: scores computed transposed T[d, qi] so softmax denominator is a
matmul-reduction; exp on ScalarE; per-head a accumulated in PSUM with an
appended ones-row in v^T producing the softmax denominator for free.
Heads are packed at 32-aligned partition quadrants (hardware requires engine
SBUF access patterns to start at partitions 0/32/64/96), split into even/odd
parity groups; gate weights are permuted/zero-padded on the host to match.
"""

import os
import numpy as np

os.environ.setdefault("MYCRO_LOCAL_CACHE", "1")

# ---- problem constants (hardcoded per contract) ----
N_BATCH = 8
HW = 36 * 36            # 1296 spatial tokens
HP = 38                 # padded spatial edge
NC_CORES = 8
QI = [(0, 512), (512, 512), (1024, 272)]          # token tiles
RB = [(0, 12), (12, 12), (24, 12)]                # conv row blocks (rows of 36)
DT = [(t * 128, 128) for t in range(10)] + [(1280, 16)]   # key/d tiles
# heads at 32-aligned quadrants: parity p holds heads [p, p+2, p+4, p+6]
# mixed-parity groups of <=3 give ScalarE exp tiles of 3 PSUM banks
GROUPS = [[(0, 0), (0, 1), (0, 2)], [(0, 3), (1, 0), (1, 1)], [(1, 2), (1, 3)]]

TRACE = False           # set by test harness for profiling
_PROG = None


def _build():
    import concourse.bacc as bacc
    import concourse.mybir as mybir
    import concourse.tile as tile
    from contextlib import ExitStack

    F32 = mybir.dt.float32
    F32R = mybir.dt.float32r
    BF16 = mybir.dt.bfloat16
    AF = mybir.ActivationFunctionType
    ALU = mybir.AluOpType

    nc = bacc.Bacc("TRN2", target_bir_lowering=False, debug=False)

    d_x = nc.dram_tensor("x", [128, HW], F32, kind="ExternalInput")
    d_w_inT = nc.dram_tensor("w_inT", [128, 128], F32, kind="ExternalInput")
    d_wc = nc.dram_tensor("wc", [9, 128, 128], F32R, kind="ExternalInput")
    d_wqT = nc.dram_tensor("wqT", [2, 128, 128], F32R, kind="ExternalInput")
    d_wkT = nc.dram_tensor("wkT", [2, 128, 128], F32R, kind="ExternalInput")
    d_wvT = nc.dram_tensor("wvT", [128, 128], F32R, kind="ExternalInput")
    d_wgzT = nc.dram_tensor("wgzT", [128, 3, 128], F32R, kind="ExternalInput")
    d_wgaT = nc.dram_tensor("wgaT", [2, 128, 3, 128], F32R, kind="ExternalInput")
    d_woutT = nc.dram_tensor("woutT", [128, 128], F32R, kind="ExternalInput")
    d_b_in = nc.dram_tensor("b_in", [128, 1], F32, kind="ExternalInput")
    d_b_conv = nc.dram_tensor("b_conv", [128, 1], F32, kind="ExternalInput")
    d_b_gates = nc.dram_tensor("b_gates", [128, 3], F32, kind="ExternalInput")
    d_b_out = nc.dram_tensor("b_out", [128, 1], F32, kind="ExternalInput")
    d_out = nc.dram_tensor("out", [128, HW], F32, kind="ExternalOutput")

    with tile.TileContext(nc) as tc, ExitStack() as ctx:
        wp = ctx.enter_context(tc.tile_pool(name="w", bufs=1))
        dp = ctx.enter_context(tc.tile_pool(name="d", bufs=1))

        def load(pool, dram, shape, tag, ap=None, dt=F32):
            t = pool.tile(shape, dt, tag=tag, name=tag)
            nc.sync.dma_start(t[:], dram[:] if ap is None else ap)
            return t

        xs = load(dp, d_x, [128, HW], "xs")
        w_inT = load(wp, d_w_inT, [128, 128], "w_inT")
        wc = load(wp, d_wc, [128, 9, 128], "wc", d_wc.ap().rearrange("t i o -> i t o"), dt=F32R)
        wqT = load(wp, d_wqT, [128, 2, 128], "wqT", d_wqT.ap().rearrange("p i o -> i p o"), dt=F32R)
        wkT = load(wp, d_wkT, [128, 2, 128], "wkT", d_wkT.ap().rearrange("p i o -> i p o"), dt=F32R)
        wvT = load(wp, d_wvT, [128, 128], "wvT", dt=F32R)
        wgzT = load(wp, d_wgzT, [128, 3, 128], "wgzT", dt=F32R)
        wgaT = load(wp, d_wgaT, [128, 2, 3, 128], "wgaT",
                    d_wgaT.ap().rearrange("p i g o -> i p g o"), dt=F32R)
        woutT = load(wp, d_woutT, [128, 128], "woutT", dt=F32R)
        b_in = load(wp, d_b_in, [128, 1], "b_in")
        b_conv = load(wp, d_b_conv, [128, 1], "b_conv")
        b_gates = load(wp, d_b_gates, [128, 3], "b_gates")
        b_out = load(wp, d_b_out, [128, 1], "b_out")
        stage0 = wp.tile([128, 1444], F32, tag="stage0")
        nc.vector.memset(stage0[:], 0.0)
        stage1 = wp.tile([128, 88], F32, tag="stage1")
        nc.vector.memset(stage1[:], 1.0)
        zc = wp.tile([1, 128], F32R, tag="zc")
        nc.vector.tensor_copy(zc[:], stage0[0:1, 0:128])

        # ---------- phase 1: xt = tanh(w_in x), z = conv3x3 + b ----------
        xt_pad = dp.tile([128, HP * HP], F32R, tag="xt_pad")
        nc.vector.tensor_copy(xt_pad[:], stage0[:, 0:HP * HP])
        xt3 = xt_pad[:].rearrange("p (h w) -> p h w", h=HP)
        z_sb = dp.tile([128, HW], F32R, tag="z_sb")
        z3 = z_sb[:].rearrange("p (h w) -> p h w", h=36)

        with tc.tile_pool(name="ps1", bufs=3, space="PSUM") as ps1:
            for (r0, nr) in RB:
                nn = nr * 36
                pp = ps1.tile([128, 512], F32, tag="ps1")
                nc.tensor.matmul(pp[:, :nn], w_inT[:],
                                 xs[:, r0 * 36: r0 * 36 + nn],
                                 start=True, stop=True)
                nc.scalar.activation(
                    xt3[:, 1 + r0: 1 + r0 + nr, 1:37],
                    pp[:, :nn].rearrange("p (h w) -> p h w", h=nr),
                    AF.Tanh, bias=b_in[:, 0:1])
            for (r0, nr) in RB:
                nn = nr * 36
                pp = ps1.tile([128, 512], F32, tag="ps1")
                for t9 in range(9):
                    dy, dx = divmod(t9, 3)
                    nc.tensor.matmul(pp[:, :nn], wc[:, t9, :],
                                     xt3[:, r0 + dy: r0 + dy + nr, dx: dx + 36],
                                     start=(t9 == 0), stop=(t9 == 8))
                nc.vector.tensor_scalar_add(
                    z3[:, r0: r0 + nr, :],
                    pp[:, :nn].rearrange("p (h w) -> p h w", h=nr),
                    b_conv[:, 0:1])

            # q/k projections into parity-quadrant layout
            q_sb = dp.tile([128, 2, HW], F32R, tag="q_sb")
            k_sb = dp.tile([128, 2, HW], F32R, tag="k_sb")
            for (q0, nq) in QI:
                for p in (0, 1):
                    for dst, wT in ((q_sb, wqT), (k_sb, wkT)):
                        pp = ps1.tile([128, 512], F32, tag="ps1")
                        nc.tensor.matmul(pp[:, :nq], wT[:, p, :],
                                         z_sb[:, q0:q0 + nq], start=True, stop=True)
                        nc.vector.tensor_copy(dst[:, p, q0:q0 + nq], pp[:, :nq])

            # vT[d, head, c]: col 0 = ones (softmax denominator), cols 1..16
            # = v^T, cols 17..31 zero (ISA needs full-32 col groups in mm2)
            vT = dp.tile([128, 11, 8, 32], BF16, tag="vT")
            vTf = vT[:].rearrange("p a b c -> p (a b c)")
            nc.vector.tensor_copy(vTf[:, 0:1408], stage0[:, 0:1408])
            nc.vector.tensor_copy(vTf[:, 1408:2816], stage0[:, 0:1408])
            nc.vector.tensor_copy(vT[:, :, :, 0:1],
                                  stage1[:].rearrange("p (a b c) -> p a b c",
                                                      a=11, b=8))
            for t, (d0, nd) in enumerate(DT):
                pp = ps1.tile([128, 512], F32, tag="ps1")
                nc.tensor.matmul(pp[0:nd, 0:128], z_sb[:, d0:d0 + nd],
                                 wvT[:], start=True, stop=True)
                nc.vector.tensor_copy(
                    vT[0:nd, t, :, 1:17],
                    pp[0:nd, 0:128].rearrange("p (h c) -> p h c", h=8))

        # ---------- phase 2: attention ----------
        a_par = dp.tile([128, 2, HW], F32R, tag="a_par")

        with tc.tile_pool(name="psT", bufs=2, space="PSUM") as psT, \
             tc.tile_pool(name="psA", bufs=1, space="PSUM") as psA, \
             tc.tile_pool(name="ep", bufs=3) as ep, \
             tc.tile_pool(name="np_", bufs=4) as npool:
            # last (g, gi) per parity for stop flags
            last_pj = {}
            for g, grp in enumerate(GROUPS):
                for gi, (p, j) in enumerate(grp):
                    last_pj[p] = (g, gi)
            for (q0, nq) in QI:
                ab = {p: psA.tile([128, 512], F32, tag=f"a{p}", name=f"a{p}") for p in (0, 1)}
                # clear each accumulator bank once (avoids concurrent
                # whole-bank has_written clears from col-packed matmuls)
                for p in (0, 1):
                    nc.tensor.matmul(ab[p][0:128, 0:nq], zc[:],
                                     z_sb[0:1, q0:q0 + nq],
                                     start=True, stop=False, skip_group_check=True)
                for g, grp in enumerate(GROUPS):
                    for t, (d0, nd) in enumerate(DT):
                        Tt = psT.tile([128, 3, 512], F32, tag="T")
                        for gi, (p, j) in enumerate(grp):
                            nc.tensor.matmul(
                                Tt[0:nd, gi, 0:nq],
                                k_sb[32 * j:32 * j + 16, p, d0:d0 + nd],
                                q_sb[32 * j:32 * j + 16, p, q0:q0 + nq],
                                start=True, stop=True, tile_position=(32 * j, 0))
                        Et = ep.tile([128, 3, 512], BF16, tag="E")
                        nc.scalar.activation(Et[0:nd, 0:len(grp), 0:nq],
                                             Tt[0:nd, 0:len(grp), 0:nq], AF.Exp)
                        for gi, (p, j) in enumerate(grp):
                            h = 2 * j + p
                            stop = (last_pj[p] == (g, gi) and t == 10)
                            nc.tensor.matmul(
                                ab[p][32 * j:32 * j + 32, 0:nq],
                                vT[0:nd, t, h, :], Et[0:nd, gi, 0:nq],
                                start=False, stop=stop, skip_group_check=True,
                                tile_position=(0, 32 * j))
                # normalize: a = a_un * (1/colsum), into a_par quadrants
                for p in (0, 1):
                    for j in range(4):
                        # row 32j of the accumulator is the colsum (ones
                        # column sits first in vT_aug); rows 32j+1..+16 = a_un
                        rec = npool.tile([1, 512], F32, tag="rec")
                        nc.vector.reciprocal(rec[0:1, 0:nq],
                                             ab[p][32 * j:32 * j + 1, 0:nq])
                        rb = npool.tile([32, 512], F32, tag="rb")
                        nc.gpsimd.partition_broadcast(rb[0:32, 0:nq], rec[0:1, 0:nq])
                        nc.vector.tensor_tensor(
                            a_par[32 * j:32 * j + 32, p, q0:q0 + nq],
                            ab[p][32 * j:32 * j + 32, 0:nq], rb[0:32, 0:nq],
                            op=ALU.mult)

        # ---------- phase 3: gates, cell update, output ----------
        with tc.tile_pool(name="psG", bufs=4, space="PSUM") as psG, \
             tc.tile_pool(name="gp", bufs=2) as gp:
            for (q0, nq) in QI:
                acts = []
                for gidx, fn in ((0, AF.Sigmoid), (1, AF.Tanh), (2, AF.Sigmoid)):
                    pp = psG.tile([128, 512], F32, tag="psg")
                    nc.tensor.matmul(pp[:, :nq], wgzT[:, gidx, :],
                                     z_sb[:, q0:q0 + nq], start=True, stop=False)
                    for p in (0, 1):
                        nc.tensor.matmul(pp[:, :nq], wgaT[:, p, gidx, :],
                                         a_par[:, p, q0:q0 + nq],
                                         start=False, stop=(p == 1))
                    gs = gp.tile([128, 512], F32, tag=f"g{gidx}")
                    nc.scalar.activation(gs[:, :nq], pp[:, :nq], fn,
                                         bias=b_gates[:, gidx:gidx + 1])
                    acts.append(gs)
                i_s, g_s, o_s = acts
                c_s = gp.tile([128, 512], F32, tag="c")
                nc.vector.tensor_tensor(c_s[:, :nq], i_s[:, :nq], g_s[:, :nq],
                                        op=ALU.mult)
                tc_s = gp.tile([128, 512], F32, tag="tc")
                nc.scalar.activation(tc_s[:, :nq], c_s[:, :nq], AF.Tanh)
                h_s = gp.tile([128, 512], F32R, tag="h")
                nc.vector.tensor_tensor(h_s[:, :nq], o_s[:, :nq], tc_s[:, :nq],
                                        op=ALU.mult)
                pp = psG.tile([128, 512], F32, tag="psg")
                nc.tensor.matmul(pp[:, :nq], woutT[:], h_s[:, :nq],
                                 start=True, stop=True)
                y_s = gp.tile([128, 512], F32, tag="y")
                nc.vector.tensor_scalar_add(y_s[:, :nq], pp[:, :nq], b_out[:, 0:1])
                nc.sync.dma_start(d_out[:, q0:q0 + nq], y_s[:, :nq])

    nc.compile()
    return nc


def _prep_weights(inputs):
    f = np.float32
    out = {}
    out["w_inT"] = np.ascontiguousarray(inputs["w_in"].T, dtype=f)
    wcv = np.asarray(inputs["w_conv"], dtype=f)[:, :128]        # [128,128,3,3]
    out["wc"] = np.ascontiguousarray(wcv.transpose(2, 3, 1, 0).reshape(9, 128, 128))

    def pack_qk(w):
        t = np.zeros((2, 128, 128), f)
        for p in range(2):
            for j in range(4):
                h = 2 * j + p
                t[p][:, 32 * j:32 * j + 16] = w[16 * h:16 * h + 16, :].T
        return t

    out["wqT"] = pack_qk(np.asarray(inputs["wq"], dtype=f))
    out["wkT"] = pack_qk(np.asarray(inputs["wk"], dtype=f))
    out["wvT"] = np.ascontiguousarray(np.asarray(inputs["wv"], dtype=f).T)
    gates = [np.asarray(inputs[k], dtype=f) for k in ("w_i", "w_g", "w_o")]
    out["wgzT"] = np.ascontiguousarray(
        np.stack([g[:, :128].T for g in gates], axis=1))
    wga = np.zeros((2, 128, 3, 128), f)
    for gi, g in enumerate(gates):
        wa = g[:, 128:]                                          # [r, ac]
        for p in range(2):
            for j in range(4):
                h = 2 * j + p
                wga[p, 32 * j + 1:32 * j + 17, gi, :] = wa[:, 16 * h:16 * h + 16].T
    out["wgaT"] = np.ascontiguousarray(wga)
    out["woutT"] = np.ascontiguousarray(np.asarray(inputs["w_out"], dtype=f).T)
    out["b_in"] = np.ascontiguousarray(np.asarray(inputs["b_in"], f).reshape(128, 1))
    out["b_conv"] = np.ascontiguousarray(np.asarray(inputs["b_conv"], f).reshape(128, 1))
    out["b_gates"] = np.ascontiguousarray(
        np.stack([np.asarray(inputs[k], f) for k in ("b_i", "b_g", "b_o")], axis=1))
    out["b_out"] = np.ascontiguousarray(np.asarray(inputs["b_out"], f).reshape(128, 1))
    return out


def kernel(**inputs):
    global _PROG
    if _PROG is None:
        _PROG = _build()
    from concourse import bass_utils

    shared = _prep_weights(inputs)
    x = np.asarray(inputs["x"], dtype=np.float32).reshape(N_BATCH, 128, HW)
    in_maps = [dict(shared, x=np.ascontiguousarray(x[n])) for n in range(N_BATCH)]
    res = bass_utils.run_bass_kernel_spmd(
        _PROG, in_maps, core_ids=list(range(NC_CORES)), trace=TRACE)
    if TRACE and res.exec_time_ns is not None:
        kernel.last_exec_time_ns = res.exec_time_ns
    out = np.stack([res.results[n]["out"] for n in range(N_BATCH)])
    return out.reshape(N_BATCH, 128, 36, 36).astype(inputs["x"].dtype, copy=False)


kernel.last_exec_time_ns = None


# revision 10
# speedup vs baseline: 1.0059x; 1.0059x over previous
"""Trainium2 Bass kernel: single-step ConvLSTM cell with spatial multi-head
attention (nn_CvtLstm). Data-parallel over batch N=8 across 8 NeuronCores.

Math (first timestep, h0 = c0 = 0):
  xt = tanh(w_in @ x + b_in)
  z  = conv3x3(xt, w_conv[:, :128]) + b_conv        (h-half of w_conv sees zeros)
  q/k/v = 1x1 projections of z, 8 heads x 16 ch over 1296 spatial tokens
  a  = per-head V @ softmax(q^T k, axis=keys)^T
  i, g, o = gates on [z; a]  (f-gate unused: c0 = 0)
  out = w_out @ (sigmoid(o) * tanh(sigmoid(i) * tanh(g))) + b_out

Layout: feature maps [channels=128 partitions, HW=1296 free].  Scores are
computed transposed, T[d, qi], per head; exp on ScalarE; a-unnormalized is
accumulated in PSUM via matmuls whose stationary v^T carries a leading ones
column, so the softmax denominator falls out of the same accumulation
(row 32j of the quadrant).  Heads live at 32-aligned partition quadrants
(engine APs must start at partitions 0/32/64/96), split into even/odd parity
tensors; gate/projection weights are permuted + zero-padded on the host to
match.  Sigmoid/tanh of the gates are computed from exp + DVE reciprocal so
the whole kernel uses one ScalarE table set and gates interleave with
attention.  Matmul operands are float32r (full-rate PE) except the
attention-output matmul, which is bf16 (walrus rejects fp32r matmuls with
non-zero dst partition).
"""

import os
import numpy as np

os.environ.setdefault("MYCRO_LOCAL_CACHE", "1")

# ---- problem constants (hardcoded per contract) ----
N_BATCH = 8
HW = 36 * 36            # 1296 spatial tokens
HP = 38                 # padded spatial edge
NC_CORES = 8
QI = [(0, 512), (512, 512), (1024, 272)]          # token tiles
RB = [(0, 12), (12, 12), (24, 12)]                # conv row blocks (rows of 36)
DT = [(t * 128, 128) for t in range(10)] + [(1280, 16)]   # key/d tiles

TRACE = False           # set by test harness for profiling
_PROG = None


def _build():
    import concourse.bacc as bacc
    import concourse.mybir as mybir
    import concourse.tile as tile
    from contextlib import ExitStack

    F32 = mybir.dt.float32
    F32R = mybir.dt.float32r
    BF16 = mybir.dt.bfloat16
    AF = mybir.ActivationFunctionType
    ALU = mybir.AluOpType

    nc = bacc.Bacc("TRN2", target_bir_lowering=False, debug=False)

    d_x = nc.dram_tensor("x", [128, HW], F32, kind="ExternalInput")
    d_w_inT = nc.dram_tensor("w_inT", [128, 128], F32, kind="ExternalInput")
    d_wc = nc.dram_tensor("wc", [9, 128, 128], F32R, kind="ExternalInput")
    d_wqT = nc.dram_tensor("wqT", [2, 128, 128], F32R, kind="ExternalInput")
    d_wkT = nc.dram_tensor("wkT", [2, 128, 128], F32R, kind="ExternalInput")
    d_wvT = nc.dram_tensor("wvT", [128, 128], F32R, kind="ExternalInput")
    d_wgzT = nc.dram_tensor("wgzT", [128, 3, 128], F32R, kind="ExternalInput")
    d_wgaT = nc.dram_tensor("wgaT", [2, 128, 3, 128], F32R, kind="ExternalInput")
    d_woutT = nc.dram_tensor("woutT", [128, 128], F32R, kind="ExternalInput")
    d_b_in = nc.dram_tensor("b_in", [128, 1], F32, kind="ExternalInput")
    d_b_conv = nc.dram_tensor("b_conv", [128, 1], F32, kind="ExternalInput")
    # negated / scaled for exp-form gates: [-b_i, -2*b_g, -b_o]
    d_b_gates = nc.dram_tensor("b_gates", [128, 3], F32, kind="ExternalInput")
    d_b_out = nc.dram_tensor("b_out", [128, 1], F32, kind="ExternalInput")
    d_out = nc.dram_tensor("out", [128, HW], F32, kind="ExternalOutput")

    GSCALE = (-1.0, -2.0, -1.0)   # exp(scale*pre + bias) per gate i, g, o

    with tile.TileContext(nc) as tc, ExitStack() as ctx:
        wp = ctx.enter_context(tc.tile_pool(name="w", bufs=1))
        dp = ctx.enter_context(tc.tile_pool(name="d", bufs=1))

        def load(pool, dram, shape, tag, ap=None, dt=F32):
            t = pool.tile(shape, dt, tag=tag, name=tag)
            nc.sync.dma_start(t[:], dram[:] if ap is None else ap)
            return t

        xs = load(dp, d_x, [128, HW], "xs")
        w_inT = load(wp, d_w_inT, [128, 128], "w_inT")
        wc = load(wp, d_wc, [128, 9, 128], "wc",
                  d_wc.ap().rearrange("t i o -> i t o"), dt=F32R)
        wqT = load(wp, d_wqT, [128, 2, 128], "wqT",
                   d_wqT.ap().rearrange("p i o -> i p o"), dt=F32R)
        wkT = load(wp, d_wkT, [128, 2, 128], "wkT",
                   d_wkT.ap().rearrange("p i o -> i p o"), dt=F32R)
        wvT = load(wp, d_wvT, [128, 128], "wvT", dt=F32R)
        wgzT = load(wp, d_wgzT, [128, 3, 128], "wgzT", dt=F32R)
        wgaT = load(wp, d_wgaT, [128, 2, 3, 128], "wgaT",
                    d_wgaT.ap().rearrange("p i g o -> i p g o"), dt=F32R)
        woutT = load(wp, d_woutT, [128, 128], "woutT", dt=F32R)
        b_in = load(wp, d_b_in, [128, 1], "b_in")
        b_conv = load(wp, d_b_conv, [128, 1], "b_conv")
        b_gates = load(wp, d_b_gates, [128, 3], "b_gates")
        b_out = load(wp, d_b_out, [128, 1], "b_out")

        # fp32 staging for float32r/bf16 constants (memset can't encode them)
        stage0 = wp.tile([128, 1444], F32, tag="stage0")
        nc.vector.memset(stage0[:], 0.0)
        stage1 = wp.tile([128, 88], F32, tag="stage1")
        nc.vector.memset(stage1[:], 1.0)
        zc = wp.tile([1, 128], F32R, tag="zc")
        nc.vector.tensor_copy(zc[:], stage0[0:1, 0:128])

        # ---------- phase 1: xt = tanh(w_in x + b_in), z = conv3x3 + b ----------
        xt_pad = dp.tile([128, HP * HP], F32R, tag="xt_pad")
        nc.vector.tensor_copy(xt_pad[:], stage0[:, 0:HP * HP])
        xt3 = xt_pad[:].rearrange("p (h w) -> p h w", h=HP)
        z_sb = dp.tile([128, HW], F32R, tag="z_sb")
        z3 = z_sb[:].rearrange("p (h w) -> p h w", h=36)

        with tc.tile_pool(name="ps1", bufs=3, space="PSUM") as ps1:
            for (r0, nr) in RB:
                nn = nr * 36
                pp = ps1.tile([128, 512], F32, tag="ps1")
                nc.tensor.matmul(pp[:, :nn], w_inT[:],
                                 xs[:, r0 * 36: r0 * 36 + nn],
                                 start=True, stop=True)
                nc.scalar.activation(
                    xt3[:, 1 + r0: 1 + r0 + nr, 1:37],
                    pp[:, :nn].rearrange("p (h w) -> p h w", h=nr),
                    AF.Tanh, bias=b_in[:, 0:1])
            for (r0, nr) in RB:
                nn = nr * 36
                pp = ps1.tile([128, 512], F32, tag="ps1")
                for t9 in range(9):
                    dy, dx = divmod(t9, 3)
                    nc.tensor.matmul(pp[:, :nn], wc[:, t9, :],
                                     xt3[:, r0 + dy: r0 + dy + nr, dx: dx + 36],
                                     start=(t9 == 0), stop=(t9 == 8))
                nc.vector.tensor_scalar_add(
                    z3[:, r0: r0 + nr, :],
                    pp[:, :nn].rearrange("p (h w) -> p h w", h=nr),
                    b_conv[:, 0:1])

            # q/k projections into parity-quadrant layout
            q_sb = dp.tile([128, 2, HW], F32R, tag="q_sb")
            k_sb = dp.tile([128, 2, HW], F32R, tag="k_sb")
            for (q0, nq) in QI:
                for p in (0, 1):
                    for dst, wT in ((q_sb, wqT), (k_sb, wkT)):
                        pp = ps1.tile([128, 512], F32, tag="ps1")
                        nc.tensor.matmul(pp[:, :nq], wT[:, p, :],
                                         z_sb[:, q0:q0 + nq], start=True, stop=True)
                        nc.vector.tensor_copy(dst[:, p, q0:q0 + nq], pp[:, :nq])

            # vT[d, head, c]: col 0 = ones (softmax denominator), cols 1..16
            # = v^T, cols 17..31 zero (matmul dst needs full 32-col groups)
            vT = dp.tile([128, 11, 8, 32], BF16, tag="vT")
            vTf = vT[:].rearrange("p a b c -> p (a b c)")
            nc.vector.tensor_copy(vTf[:, 0:1408], stage0[:, 0:1408])
            nc.vector.tensor_copy(vTf[:, 1408:2816], stage0[:, 0:1408])
            nc.vector.tensor_copy(vT[:, :, :, 0:1],
                                  stage1[:].rearrange("p (a b c) -> p a b c",
                                                      a=11, b=8))
            for t, (d0, nd) in enumerate(DT):
                pp = ps1.tile([128, 512], F32, tag="ps1")
                nc.tensor.matmul(pp[0:nd, 0:128], z_sb[:, d0:d0 + nd],
                                 wvT[:], start=True, stop=True)
                nc.vector.tensor_copy(
                    vT[0:nd, t, :, 1:17],
                    pp[0:nd, 0:128].rearrange("p (h c) -> p h c", h=8))

        # ---------- phase 2+3: attention with inline gates ----------
        a_par = dp.tile([128, 2, HW], F32R, tag="a_par")

        with tc.tile_pool(name="psT", bufs=2, space="PSUM") as psT, \
             tc.tile_pool(name="psA", bufs=2, space="PSUM") as psA, \
             tc.tile_pool(name="psG", bufs=2, space="PSUM") as psG, \
             tc.tile_pool(name="ep", bufs=5) as ep, \
             tc.tile_pool(name="np_", bufs=6) as npool, \
             tc.tile_pool(name="gp", bufs=2) as gp:
            for (q0, nq) in QI:
                for p in (0, 1):
                    ab = psA.tile([128, 512], F32, tag="a", name=f"a{p}")
                    # zero the whole accumulator bank once (single matmul
                    # owns start=True; avoids concurrent whole-bank
                    # has_written clears from col-packed accumulation)
                    nc.tensor.matmul(ab[0:128, 0:nq], zc[:],
                                     z_sb[0:1, q0:q0 + nq],
                                     start=True, stop=False, skip_group_check=True)
                    for jj in ((0, 1), (2, 3)):
                        for t, (d0, nd) in enumerate(DT):
                            Tt = psT.tile([128, 2, 512], F32, tag="T")
                            for gi, j in enumerate(jj):
                                nc.tensor.matmul(
                                    Tt[0:nd, gi, 0:nq],
                                    k_sb[32 * j:32 * j + 16, p, d0:d0 + nd],
                                    q_sb[32 * j:32 * j + 16, p, q0:q0 + nq],
                                    start=True, stop=True,
                                    tile_position=(32 * j, 0))
                            Et = ep.tile([128, 2, 512], BF16, tag="E")
                            nc.scalar.activation(Et[0:nd, :, 0:nq],
                                                 Tt[0:nd, :, 0:nq], AF.Exp)
                            for gi, j in enumerate(jj):
                                h = 2 * j + p
                                stop = (jj[1] == 3 and gi == 1 and t == 10)
                                nc.tensor.matmul(
                                    ab[32 * j:32 * j + 32, 0:nq],
                                    vT[0:nd, t, h, :], Et[0:nd, gi, 0:nq],
                                    start=False, stop=stop, skip_group_check=True,
                                    tile_position=(0, 32 * j))
                    # normalize: quadrant row 32j = colsum, rows +1..+16 = a_un
                    for j in range(4):
                        rec = npool.tile([1, 512], F32, tag="rec")
                        nc.vector.reciprocal(rec[0:1, 0:nq],
                                             ab[32 * j:32 * j + 1, 0:nq])
                        rb = npool.tile([32, 512], F32, tag="rb")
                        nc.gpsimd.partition_broadcast(rb[0:32, 0:nq], rec[0:1, 0:nq])
                        nc.vector.tensor_tensor(
                            a_par[32 * j:32 * j + 32, p, q0:q0 + nq],
                            ab[32 * j:32 * j + 32, 0:nq], rb[0:32, 0:nq],
                            op=ALU.mult)

                # gates for this token tile (exp-form; shares the exp table)
                acts = []
                for gidx in range(3):
                    pp = psG.tile([128, 512], F32, tag="psg")
                    nc.tensor.matmul(pp[:, :nq], wgzT[:, gidx, :],
                                     z_sb[:, q0:q0 + nq], start=True, stop=False)
                    for p in (0, 1):
                        nc.tensor.matmul(pp[:, :nq], wgaT[:, p, gidx, :],
                                         a_par[:, p, q0:q0 + nq],
                                         start=False, stop=(p == 1))
                    # e = exp(scale*pre + bias);  sig = 1/(1+e);  tanh = 2*sig-1
                    es = gp.tile([128, 512], F32, tag=f"e{gidx}")
                    nc.scalar.activation(es[:, :nq], pp[:, :nq], AF.Exp,
                                         bias=b_gates[:, gidx:gidx + 1],
                                         scale=GSCALE[gidx])
                    nc.vector.tensor_scalar_add(es[:, :nq], es[:, :nq], 1.0)
                    rs = gp.tile([128, 512], F32, tag=f"r{gidx}")
                    nc.vector.reciprocal(rs[:, :nq], es[:, :nq])
                    acts.append(rs)
                i_r, g_r, o_r = acts
                # c = sigmoid(i) * tanh(g) = i_r * (2*g_r - 1)
                g_t = gp.tile([128, 512], F32, tag="g_t")
                nc.vector.tensor_scalar(g_t[:, :nq], g_r[:, :nq], 2.0, -1.0,
                                        op0=ALU.mult, op1=ALU.add)
                c_s = gp.tile([128, 512], F32, tag="c")
                nc.vector.tensor_tensor(c_s[:, :nq], i_r[:, :nq], g_t[:, :nq],
                                        op=ALU.mult)
                # tanh(c) = 2/(1+exp(-2c)) - 1
                e2 = gp.tile([128, 512], F32, tag="e2")
                nc.scalar.activation(e2[:, :nq], c_s[:, :nq], AF.Exp, scale=-2.0)
                nc.vector.tensor_scalar_add(e2[:, :nq], e2[:, :nq], 1.0)
                r2 = gp.tile([128, 512], F32, tag="r2")
                nc.vector.reciprocal(r2[:, :nq], e2[:, :nq])
                nc.vector.tensor_scalar(r2[:, :nq], r2[:, :nq], 2.0, -1.0,
                                        op0=ALU.mult, op1=ALU.add)
                # h = sigmoid(o) * tanh(c)
                h_s = gp.tile([128, 512], F32R, tag="h")
                nc.vector.tensor_tensor(h_s[:, :nq], o_r[:, :nq], r2[:, :nq],
                                        op=ALU.mult)
                pp = psG.tile([128, 512], F32, tag="psg")
                nc.tensor.matmul(pp[:, :nq], woutT[:], h_s[:, :nq],
                                 start=True, stop=True)
                y_s = gp.tile([128, 512], F32, tag="y")
                nc.vector.tensor_scalar_add(y_s[:, :nq], pp[:, :nq], b_out[:, 0:1])
                nc.sync.dma_start(d_out[:, q0:q0 + nq], y_s[:, :nq])

    nc.compile()
    return nc


def _prep_weights(inputs):
    f = np.float32
    out = {}
    out["w_inT"] = np.ascontiguousarray(inputs["w_in"].T, dtype=f)
    wcv = np.asarray(inputs["w_conv"], dtype=f)[:, :128]        # [128,128,3,3]
    out["wc"] = np.ascontiguousarray(wcv.transpose(2, 3, 1, 0).reshape(9, 128, 128))

    def pack_qk(w):
        t = np.zeros((2, 128, 128), f)
        for p in range(2):
            for j in range(4):
                h = 2 * j + p
                t[p][:, 32 * j:32 * j + 16] = w[16 * h:16 * h + 16, :].T
        return t

    out["wqT"] = pack_qk(np.asarray(inputs["wq"], dtype=f))
    out["wkT"] = pack_qk(np.asarray(inputs["wk"], dtype=f))
    out["wvT"] = np.ascontiguousarray(np.asarray(inputs["wv"], dtype=f).T)
    gates = [np.asarray(inputs[k], dtype=f) for k in ("w_i", "w_g", "w_o")]
    out["wgzT"] = np.ascontiguousarray(
        np.stack([g[:, :128].T for g in gates], axis=1))
    wga = np.zeros((2, 128, 3, 128), f)
    for gi, g in enumerate(gates):
        wa = g[:, 128:]                                          # [r, ac]
        for p in range(2):
            for j in range(4):
                h = 2 * j + p
                # a-channel c of head h lives at partition 32j+1+c
                wga[p, 32 * j + 1:32 * j + 17, gi, :] = wa[:, 16 * h:16 * h + 16].T
    out["wgaT"] = np.ascontiguousarray(wga)
    out["woutT"] = np.ascontiguousarray(np.asarray(inputs["w_out"], dtype=f).T)
    out["b_in"] = np.ascontiguousarray(np.asarray(inputs["b_in"], f).reshape(128, 1))
    out["b_conv"] = np.ascontiguousarray(np.asarray(inputs["b_conv"], f).reshape(128, 1))
    # exp-form gate biases: exp(scale*(pre + b)) -> bias' = scale * b
    out["b_gates"] = np.ascontiguousarray(
        np.stack([-np.asarray(inputs["b_i"], f),
                  -2.0 * np.asarray(inputs["b_g"], f),
                  -np.asarray(inputs["b_o"], f)], axis=1))
    out["b_out"] = np.ascontiguousarray(np.asarray(inputs["b_out"], f).reshape(128, 1))
    return out


def kernel(**inputs):
    global _PROG
    if _PROG is None:
        _PROG = _build()
    from concourse import bass_utils

    shared = _prep_weights(inputs)
    x = np.asarray(inputs["x"], dtype=np.float32).reshape(N_BATCH, 128, HW)
    in_maps = [dict(shared, x=np.ascontiguousarray(x[n])) for n in range(N_BATCH)]
    res = bass_utils.run_bass_kernel_spmd(
        _PROG, in_maps, core_ids=list(range(NC_CORES)), trace=TRACE)
    if TRACE and res.exec_time_ns is not None:
        kernel.last_exec_time_ns = res.exec_time_ns
    out = np.stack([res.results[n]["out"] for n in range(N_BATCH)])
    return out.reshape(N_BATCH, 128, 36, 36).astype(inputs["x"].dtype, copy=False)


kernel.last_exec_time_ns = None
